# revision 1
# baseline (speedup 1.0000x reference)
"""KGAT recommender (3-layer GNN message passing) on 8 Trainium2 NeuronCores.

Sharding: edges are sharded by destination-node range — core k owns nodes
[k*12500, (k+1)*12500) and aggregates all messages into them, so no
all-reduce is needed; each layer ends with an AllGather of the updated
(bf16) node-embedding table (plus the per-edge attention scalar s=x@Wa_top
appended to each row so edge gathers fetch it for free).

Per 128-edge chunk the attention-weighted segment-sum is computed as a
one-hot matmul: W[e, j] = (j == dst_local[e]) * att[e] built in a single
DVE tensor_scalar op, then PSUM accumulates aggT[d, n] += G[e, d]^T @ W.
"""

import os
import numpy as np
import ml_dtypes

import concourse.bacc as bacc
import concourse.bass as bass
import concourse.mybir as mybir
import concourse.tile as tile
from concourse.bass_utils import run_bass_kernel_spmd
from concourse.masks import make_identity

BF16 = ml_dtypes.bfloat16

NCORES = 8
N = 100000
U = 50000
D = 128
L = 3
P = 128
NPC = N // NCORES          # 12500 nodes per core
WPC = (NPC + P - 1) // P   # 98 windows per core
NSLAB = WPC * P            # 12544 padded rows per core
TAB = NCORES * NSLAB       # 100352 rows in the gather table
SBW = 5                    # windows per superblock

LAST_EXEC_NS = None


def _host_prep(edge_index, user_emb, item_emb, Wa, ba, Wg, bg):
    x0 = np.concatenate([np.asarray(user_emb), np.asarray(item_emb)], 0).astype(np.float32)
    Wa = np.asarray(Wa, np.float32)
    ba = np.asarray(ba, np.float32)
    Wg = np.asarray(Wg, np.float32)
    bg = np.asarray(bg, np.float32)

    src = np.asarray(edge_index[0]).astype(np.int64)
    dst = np.asarray(edge_index[1]).astype(np.int64)
    E = src.shape[0]

    core = dst // NPC
    local = dst % NPC
    w = local // P
    dloc = local % P
    cell = core * WPC + w
    order = np.argsort(cell, kind="stable")
    cell_s = cell[order]
    counts = np.bincount(cell, minlength=NCORES * WPC)
    C = int(np.ceil(counts.max() / P))      # chunks per window (uniform)
    NCHUNK = WPC * C
    starts = np.zeros(NCORES * WPC, np.int64)
    starts[1:] = np.cumsum(counts)[:-1]
    rank = np.arange(E, dtype=np.int64) - starts[cell_s]
    k_arr = cell_s // WPC
    chunk = (cell_s % WPC) * C + rank // P
    p = rank % P

    srcs = src[order]
    idx1 = np.zeros((NCORES, P, NCHUNK), np.int32)
    idx2 = np.zeros((NCORES, P, NCHUNK), np.int32)
    dla = np.full((NCORES, P, NCHUNK), 300.0, np.float32)
    tabrow = (srcs // NPC) * NSLAB + (srcs % NPC)
    idx1[k_arr, p, chunk] = tabrow.astype(np.int32)
    idx2[k_arr, p, chunk] = local[order].astype(np.int32)
    dla[k_arr, p, chunk] = dloc[order].astype(np.float32)

    # layer-0 per-node attention scalars
    s0 = x0 @ Wa[0, :D, 0] + ba[0, 0]
    t0 = x0 @ Wa[0, D:, 0]

    xslab = np.zeros((NCORES, NSLAB, 256), BF16)
    for k in range(NCORES):
        xslab[k, :NPC, :D] = x0[k * NPC:(k + 1) * NPC].astype(BF16)
        xslab[k, :NPC, D] = s0[k * NPC:(k + 1) * NPC].astype(BF16)

    xt0 = np.zeros((NCORES, P, NSLAB), BF16)
    t0a = np.zeros((NCORES, NSLAB, 1), np.float32)
    for k in range(NCORES):
        xp = np.zeros((NSLAB, D), np.float32)
        xp[:NPC] = x0[k * NPC:(k + 1) * NPC]
        xt0[k] = np.ascontiguousarray(xp.T).astype(BF16)
        t0a[k, :NPC, 0] = t0[k * NPC:(k + 1) * NPC]

    wg_b = np.zeros((L, 2, D, D), BF16)
    for l in range(L):
        wg_b[l, 0] = Wg[l, :D].astype(BF16)
        wg_b[l, 1] = Wg[l, D:].astype(BF16)
    wast = np.zeros((L - 1, D, 2), BF16)
    for l in range(1, L):
        wast[l - 1, :, 0] = Wa[l, :D, 0].astype(BF16)
        wast[l - 1, :, 1] = Wa[l, D:, 0].astype(BF16)
    bg_c = bg.reshape(L, D, 1).astype(np.float32)

    return dict(C=C, NCHUNK=NCHUNK, idx1=idx1, idx2=idx2, dla=dla, xslab=xslab,
                xt0=xt0, t0a=t0a, wg_b=wg_b, wast=wast, bg_c=bg_c, ba=ba)


def _build_nc(C, NCHUNK, ba):
    L_RUN = int(os.environ.get("KGAT_LAYERS", str(L)))
    dt = mybir.dt
    nc = bacc.Bacc("TRN2", target_bir_lowering=False, debug=False,
                   enable_asserts=False, num_devices=NCORES)

    i_xslab = nc.dram_tensor("xslab", [NSLAB, 256], dt.bfloat16, kind="ExternalInput")
    i_xt0 = nc.dram_tensor("xt0", [P, NSLAB], dt.bfloat16, kind="ExternalInput")
    i_t0 = nc.dram_tensor("t0", [NSLAB, 1], dt.float32, kind="ExternalInput")
    i_idx1 = nc.dram_tensor("idx1", [P, NCHUNK], dt.int32, kind="ExternalInput")
    i_idx2 = nc.dram_tensor("idx2", [P, NCHUNK], dt.int32, kind="ExternalInput")
    i_dla = nc.dram_tensor("dla", [P, NCHUNK], dt.float32, kind="ExternalInput")
    i_wg = nc.dram_tensor("wg", [L, 2, D, D], dt.bfloat16, kind="ExternalInput")
    i_wast = nc.dram_tensor("wast", [L - 1, D, 2], dt.bfloat16, kind="ExternalInput")
    i_bg = nc.dram_tensor("bg", [L, D, 1], dt.float32, kind="ExternalInput")
    o_out = nc.dram_tensor("out", [NSLAB, D], dt.float32, kind="ExternalOutput")

    agin = [nc.dram_tensor(f"agin{l}", [NSLAB, 256], dt.bfloat16, kind="Internal")
            for l in range(L)]
    xfull = [nc.dram_tensor(f"xfull{l}", [TAB, 256], dt.bfloat16, kind="Internal",
                            addr_space="Shared")
             for l in range(L)]
    tbl = [nc.dram_tensor(f"tbl{l}", [NSLAB, 1], dt.float32, kind="Internal")
           for l in range(L - 1)]

    with tile.TileContext(nc) as tc:
        with (
            tc.tile_pool(name="sb", bufs=1) as sb,
            tc.tile_pool(name="sbg", bufs=2) as sbg,
            tc.tile_pool(name="sbw", bufs=3) as sbw,
            tc.tile_pool(name="ps", bufs=2, space="PSUM") as ps,
            tc.tile_pool(name="ps1", bufs=1, space="PSUM") as ps1,
        ):
            # ---- constants / persistent state ----
            iota_i = sb.tile([P, P], dt.int32)
            nc.gpsimd.iota(iota_i[:], pattern=[[1, P]], base=0, channel_multiplier=0)
            iota_f = sb.tile([P, P], dt.float32)
            nc.vector.tensor_copy(out=iota_f[:], in_=iota_i[:])
            ident_b = sb.tile([P, P], dt.bfloat16)
            make_identity(nc, ident_b[:])
            ident_f = sb.tile([P, P], dt.float32)
            make_identity(nc, ident_f[:])

            idx1_sb = sb.tile([P, NCHUNK], dt.int32)
            nc.sync.dma_start(out=idx1_sb[:], in_=i_idx1.ap())
            idx2_sb = sb.tile([P, NCHUNK], dt.int32)
            nc.sync.dma_start(out=idx2_sb[:], in_=i_idx2.ap())
            dla_sb = sb.tile([P, NCHUNK], dt.float32)
            nc.sync.dma_start(out=dla_sb[:], in_=i_dla.ap())

            wg_sb = sb.tile([P, L * 2 * D], dt.bfloat16)
            for l in range(L):
                for h in range(2):
                    nc.sync.dma_start(out=wg_sb[:, (l * 2 + h) * D:(l * 2 + h + 1) * D],
                                      in_=i_wg.ap()[l, h])
            wast_sb = sb.tile([P, (L - 1) * 2], dt.bfloat16)
            for l in range(L - 1):
                nc.sync.dma_start(out=wast_sb[:, l * 2:l * 2 + 2], in_=i_wast.ap()[l])
            bg_sb = sb.tile([P, L], dt.float32)
            for l in range(L):
                nc.sync.dma_start(out=bg_sb[:, l:l + 1], in_=i_bg.ap()[l])

            xt_own = sb.tile([P, NSLAB], dt.bfloat16)
            nc.sync.dma_start(out=xt_own[:], in_=i_xt0.ap())

            # replicate the layer-0 table: own slab -> AllGather
            nc.sync.dma_start(out=agin[0].ap(), in_=i_xslab.ap())
            nc.gpsimd.collective_compute(
                "AllGather", mybir.AluOpType.bypass,
                replica_groups=[list(range(NCORES))],
                ins=[agin[0].ap()], outs=[xfull[0].ap()])

            xsrcs = xfull
            tsrcs = [i_t0] + tbl

            for l in range(L_RUN):
                last = (l == L_RUN - 1)
                xsrc, tsrc = xsrcs[l], tsrcs[l]
                if not last:
                    stage = sb.tile([P, WPC, 256], dt.bfloat16, tag="stage")
                    nc.vector.memset(stage[:], 0)
                    tstage = sb.tile([P, WPC], dt.float32, tag="tstage")
                else:
                    stagef = sb.tile([P, WPC, D], dt.float32, tag="stage")

                maxw = int(os.environ.get("KGAT_MAXW", str(WPC)))
                w0 = 0
                while w0 < maxw:
                    w1 = min(w0 + SBW, maxw)
                    gc0, gc1 = w0 * C, w1 * C
                    SBC = gc1 - gc0
                    # one [128,1]-offset indirect gather per chunk — the
                    # multi-index form mis-lowers through neuronx_cc here
                    G = sbg.tile([P, SBC, 256], dt.bfloat16, tag="G")
                    TDt = sbg.tile([P, SBC, 1], dt.float32, tag="TD")
                    for c in range(SBC):
                        nc.gpsimd.indirect_dma_start(
                            out=G[:, c, :], out_offset=None, in_=xsrc.ap(),
                            in_offset=bass.IndirectOffsetOnAxis(
                                ap=idx1_sb[:, gc0 + c:gc0 + c + 1], axis=0))
                        nc.gpsimd.indirect_dma_start(
                            out=TDt[:, c, :], out_offset=None, in_=tsrc.ap(),
                            in_offset=bass.IndirectOffsetOnAxis(
                                ap=idx2_sb[:, gc0 + c:gc0 + c + 1], axis=0))
                    Ut = sbg.tile([P, SBC, 1], dt.float32, tag="U")
                    nc.vector.tensor_tensor(out=Ut[:], in0=TDt[:],
                                            in1=G[:, :, D:D + 1],
                                            op=mybir.AluOpType.add)
                    ATT = sbg.tile([P, SBC, 1], dt.float32, tag="ATT")
                    nc.scalar.activation(out=ATT[:], in_=Ut[:],
                                         func=mybir.ActivationFunctionType.Sigmoid)

                    aggp = None
                    for gc in range(gc0, gc1):
                        w, j, c = gc // C, gc % C, gc - gc0
                        Wt = sbw.tile([P, P], dt.bfloat16, tag="W")
                        nc.vector.tensor_scalar(
                            Wt[:], iota_f[:],
                            dla_sb[:, gc:gc + 1], ATT[:, c, 0:1],
                            mybir.AluOpType.is_equal, mybir.AluOpType.mult)
                        if j == 0:
                            aggp = ps.tile([P, P], dt.float32, tag="agg")
                        nc.tensor.matmul(out=aggp[:], lhsT=G[:, c, 0:D], rhs=Wt[:],
                                         start=(j == 0), stop=(j == C - 1))
                        if j != C - 1:
                            continue

                        # ---- window w complete: node update ----
                        aggb = sbw.tile([P, P], dt.bfloat16, tag="aggb")
                        nc.vector.tensor_copy(out=aggb[:], in_=aggp[:])
                        xts = xt_own[:, w * P:(w + 1) * P]
                        up = ps.tile([P, P], dt.float32, tag="up")
                        nc.tensor.matmul(out=up[:],
                                         lhsT=wg_sb[:, (l * 2) * D:(l * 2 + 1) * D],
                                         rhs=xts, start=True, stop=False)
                        nc.tensor.matmul(out=up[:],
                                         lhsT=wg_sb[:, (l * 2 + 1) * D:(l * 2 + 2) * D],
                                         rhs=aggb[:], start=False, stop=True)
                        if not last:
                            nc.scalar.activation(out=xts, in_=up[:],
                                                 func=mybir.ActivationFunctionType.Relu,
                                                 bias=bg_sb[:, l:l + 1])
                            st = ps1.tile([P, 2], dt.float32, tag="st")
                            nc.tensor.matmul(out=st[:], lhsT=xts,
                                             rhs=wast_sb[:, l * 2:l * 2 + 2],
                                             start=True, stop=True)
                            tr = ps1.tile([P, P], dt.bfloat16, tag="tr")
                            nc.tensor.transpose(out=tr[:], in_=xts, identity=ident_b[:])
                            nc.vector.tensor_copy(out=stage[:, w, 0:D], in_=tr[:])
                            nc.scalar.add(out=stage[:, w, D:D + 1], in_=st[:, 0:1],
                                          add=float(ba[l + 1, 0]))
                            nc.vector.tensor_copy(out=tstage[:, w:w + 1], in_=st[:, 1:2])
                        else:
                            xf = sbw.tile([P, P], dt.float32, tag="xf")
                            nc.scalar.activation(out=xf[:], in_=up[:],
                                                 func=mybir.ActivationFunctionType.Relu,
                                                 bias=bg_sb[:, l:l + 1])
                            trf = ps1.tile([P, P], dt.float32, tag="trf")
                            nc.tensor.transpose(out=trf[:], in_=xf[:], identity=ident_f[:])
                            nc.vector.tensor_copy(out=stagef[:, w, :], in_=trf[:])
                    w0 = w1

                if not last:
                    nc.sync.dma_start(
                        out=agin[l + 1].ap().rearrange("(w p) c -> p w c", p=P),
                        in_=stage[:])
                    nc.sync.dma_start(
                        out=tbl[l].ap().rearrange("(w p) o -> p (w o)", p=P),
                        in_=tstage[:])
                    nc.gpsimd.collective_compute(
                        "AllGather", mybir.AluOpType.bypass,
                        replica_groups=[list(range(NCORES))],
                        ins=[agin[l + 1].ap()], outs=[xfull[l + 1].ap()])
                else:
                    nc.sync.dma_start(
                        out=o_out.ap().rearrange("(w p) c -> p w c", p=P),
                        in_=stagef[:])

    nc.compile()
    return nc


def kernel(edge_index, user_emb, item_emb, Wa, ba, Wg, bg):
    global LAST_EXEC_NS
    h = _host_prep(edge_index, user_emb, item_emb, Wa, ba, Wg, bg)
    nc = _build_nc(h["C"], h["NCHUNK"], h["ba"])

    in_maps = []
    for k in range(NCORES):
        in_maps.append({
            "xslab": h["xslab"][k], "xt0": h["xt0"][k], "t0": h["t0a"][k],
            "idx1": h["idx1"][k], "idx2": h["idx2"][k], "dla": h["dla"][k],
            "wg": h["wg_b"], "wast": h["wast"], "bg": h["bg_c"],
        })

    res = run_bass_kernel_spmd(nc, in_maps, core_ids=list(range(NCORES)))
    LAST_EXEC_NS = res.exec_time_ns

    if int(os.environ.get("KGAT_BENCH", "0")):
        LAST_EXEC_NS = _bench(nc, in_maps)

    x = np.zeros((N, D), np.float32)
    for k in range(NCORES):
        x[k * NPC:(k + 1) * NPC] = np.asarray(res.results[k]["out"])[:NPC]
    return x[:U], x[U:]


def _bench(nc, in_maps, iters=6):
    """Time repeated on-device executions via the same PJRT shard_map path
    (device-resident inputs, no donation) and return min wall ns."""
    import time
    import jax
    from jax.sharding import Mesh, PartitionSpec
    from jax.experimental.shard_map import shard_map
    from concourse import bass2jax, mybir as mb

    bass2jax.install_neuronx_cc_hook()
    in_names, out_names, out_avals, zero_outs = [], [], [], []
    for alloc in nc.m.functions[0].allocations:
        if not isinstance(alloc, mb.MemoryLocationSet):
            continue
        name = alloc.memorylocations[0].name
        if alloc.kind == "ExternalInput":
            in_names.append(name)
        elif alloc.kind == "ExternalOutput":
            out_names.append(name)
            shape = tuple(alloc.tensor_shape)
            dtype = mb.dt.np(alloc.dtype)
            out_avals.append(jax.core.ShapedArray(shape, dtype))
            zero_outs.append(np.zeros(shape, dtype))
    n_params = len(in_names)
    all_names = in_names + out_names

    def _body(*args):
        return tuple(bass2jax._bass_exec_p.bind(
            *args, out_avals=tuple(out_avals), in_names=tuple(all_names),
            out_names=tuple(out_names), lowering_input_output_aliases=(),
            sim_require_finite=False, sim_require_nnan=False, nc=nc))

    devices = jax.devices()[:NCORES]
    mesh = Mesh(np.asarray(devices), ("core",))
    specs = (PartitionSpec("core"),) * (n_params + len(out_names))
    fn = jax.jit(shard_map(_body, mesh=mesh, in_specs=specs,
                           out_specs=(PartitionSpec("core"),) * len(out_names),
                           check_rep=False), keep_unused=True)
    concat_in = [np.concatenate([np.asarray(m[n]) for m in in_maps], axis=0)
                 for n in in_names]
    concat_zero = [np.zeros((NCORES * z.shape[0], *z.shape[1:]), z.dtype)
                   for z in zero_outs]
    sharding = jax.sharding.NamedSharding(mesh, PartitionSpec("core"))
    dev_in = [jax.device_put(a, sharding) for a in concat_in + concat_zero]
    jax.block_until_ready(fn(*dev_in))  # warm compile
    best = None
    for _ in range(iters):
        t0 = time.perf_counter()
        jax.block_until_ready(fn(*dev_in))
        dt = time.perf_counter() - t0
        best = dt if best is None else min(best, dt)
    return int(best * 1e9)



# revision 10
# speedup vs baseline: 364.4254x; 364.4254x over previous
"""KGAT recommender (3-layer GNN message passing) on 8 Trainium2 NeuronCores.

Sharding: edges are sharded by destination-node range — core k owns nodes
[k*12500, (k+1)*12500) and aggregates all messages into them; each layer ends
with an AllGather of the updated (bf16) node-embedding table (x rows carry the
per-node attention scalar s=x@Wa_top+ba at col 128, so edge gathers fetch it
for free).

The per-edge source-row gather is done with dma_gather (one Pool instruction
per (region, src-quarter) — ~4 instructions per ~80 chunks instead of one
indirect DMA per 128-edge chunk).  dma_gather indices are int16, so the
100352-row replicated table is addressed as 4 quarters of 25088 rows; edges
are grouped (window, quarter) on the host so each gather call covers a
contiguous chunk range.

Attention: att = sigmoid(s[src] + t[dst]).  t is never gathered per edge;
instead, per dst window w a rank-1 matmul (ones^T @ diag(t_w)) broadcasts t_w
across partitions into PSUM, and per chunk the scalar engine computes
SIG[e,j] = sigmoid(t_w[j] + s_e) with s_e as the per-partition activation
bias.  The DVE builds the one-hot W[e,j] = (j == dla[e]) * SIG[e,j] in two
ops, and PSUM accumulates aggT += G[:, c, 0:128]^T @ W over the window's
chunks.
"""

import os
import numpy as np
import ml_dtypes

import concourse.bacc as bacc
import concourse.bass as bass
import concourse.mybir as mybir
import concourse.tile as tile
from concourse.bass_utils import run_bass_kernel_spmd
from concourse.masks import make_identity

BF16 = ml_dtypes.bfloat16

NCORES = 8
N = 100000
U = 50000
D = 128
L = 3
P = 128
NPC = N // NCORES          # 12500 nodes per core
WPC = (NPC + P - 1) // P   # 98 windows per core
NSLAB = WPC * P            # 12544 padded rows per core
TAB = NCORES * NSLAB       # 100352 rows in the gather table
NQ = 4                     # src quarters (int16 index limit)
QROWS = TAB // NQ          # 25088 rows per quarter
ROWC = 256                 # table row: [x bf16 x128 | s bf16 | pad]

LAST_EXEC_NS = None


def _host_prep(edge_index, user_emb, item_emb, Wa, ba, Wg, bg):
    x0 = np.concatenate([np.asarray(user_emb), np.asarray(item_emb)], 0).astype(np.float32)
    Wa = np.asarray(Wa, np.float32)
    ba = np.asarray(ba, np.float32)
    Wg = np.asarray(Wg, np.float32)
    bg = np.asarray(bg, np.float32)

    src = np.asarray(edge_index[0]).astype(np.int64)
    dst = np.asarray(edge_index[1]).astype(np.int64)

    core = dst // NPC
    local = dst % NPC
    w_of = local // P
    dloc = local % P
    tabrow = (src // NPC) * NSLAB + (src % NPC)
    quarter = tabrow // QROWS
    qrow = tabrow % QROWS

    # per-(core, window, quarter) edge counts -> uniform chunk structure
    cnt = np.zeros((NCORES, WPC, NQ), np.int64)
    np.add.at(cnt, (core, w_of, quarter), 1)
    cwq = np.ceil(cnt.max(axis=0) / P).astype(np.int64)      # [WPC, NQ]
    # every window needs >= 1 chunk so its PSUM tile gets written
    for w in range(WPC):
        if cwq[w].sum() == 0:
            cwq[w, 0] = 1

    RW = int(os.environ.get("KGAT_RW", "10"))
    regions = []
    win_chunks = [[] for _ in range(WPC)]
    ch = 0
    w0 = 0
    while w0 < WPC:
        w1 = min(w0 + RW, WPC)
        ch0 = ch
        q_ranges = []
        for q in range(NQ):
            cha = ch
            for w in range(w0, w1):
                for _ in range(cwq[w, q]):
                    win_chunks[w].append(ch)
                    ch += 1
            if ch > cha:
                q_ranges.append((q, cha, ch))
        regions.append(dict(w0=w0, w1=w1, ch0=ch0, ch1=ch, q_ranges=q_ranges))
        w0 = w1
    TOTCH = ch

    # chunk-slot assignment per core
    # chunk base position per (w, q): first chunk id
    chunk_base = np.zeros((WPC, NQ), np.int64)
    for w in range(WPC):
        i = 0
        for q in range(NQ):
            chunk_base[w, q] = win_chunks[w][i] if cwq[w, q] > 0 else -1
            i += cwq[w, q]

    idx16 = np.zeros((NCORES, 16, TOTCH * 8), np.int16)
    dla = np.full((NCORES, P, TOTCH), 300.0, np.float32)
    key = (core * WPC + w_of) * NQ + quarter
    order = np.argsort(key, kind="stable")
    ks, ws, qs = core[order], w_of[order], quarter[order]
    key_s = key[order]
    starts = np.searchsorted(key_s, np.arange(NCORES * WPC * NQ))
    rank = np.arange(len(order)) - starts[key_s]
    chunk_g = chunk_base[ws, qs] + rank // P      # global chunk id
    p_slot = rank % P
    k_flat = chunk_g * P + p_slot                 # global slot index
    idx16[ks, k_flat % 16, k_flat // 16] = qrow[order].astype(np.int16)
    dla[ks, p_slot, chunk_g] = dloc[order].astype(np.float32)
    idx16 = np.tile(idx16, (1, 8, 1))             # replicate to 128 partitions

    # layer-0 per-node attention scalars
    s0 = x0 @ Wa[0, :D, 0] + ba[0, 0]
    t0 = x0 @ Wa[0, D:, 0]

    xslab = np.zeros((NCORES, NSLAB, ROWC), BF16)
    t0c = np.zeros((NCORES, P, WPC), np.float32)
    xt0 = np.zeros((NCORES, P, NSLAB), BF16)
    for k in range(NCORES):
        xslab[k, :NPC, :D] = x0[k * NPC:(k + 1) * NPC].astype(BF16)
        xslab[k, :NPC, D] = s0[k * NPC:(k + 1) * NPC].astype(BF16)
        tk = np.zeros(NSLAB, np.float32)
        tk[:NPC] = t0[k * NPC:(k + 1) * NPC]
        t0c[k] = tk.reshape(WPC, P).T
        xp = np.zeros((NSLAB, D), np.float32)
        xp[:NPC] = x0[k * NPC:(k + 1) * NPC]
        xt0[k] = np.ascontiguousarray(xp.T).astype(BF16)

    wg_b = np.zeros((L, 2, D, D), BF16)
    for l in range(L):
        wg_b[l, 0] = Wg[l, :D].astype(BF16)
        wg_b[l, 1] = Wg[l, D:].astype(BF16)
    wast = np.zeros((L - 1, D, 2), BF16)
    for l in range(1, L):
        wast[l - 1, :, 0] = Wa[l, :D, 0].astype(BF16)
        wast[l - 1, :, 1] = Wa[l, D:, 0].astype(BF16)
    bg_c = bg.reshape(L, D, 1).astype(np.float32)

    layout = dict(TOTCH=TOTCH, regions=regions, win_chunks=win_chunks)
    return dict(layout=layout, idx16=idx16, dla=dla, xslab=xslab, t0c=t0c,
                xt0=xt0, wg_b=wg_b, wast=wast, bg_c=bg_c, ba=ba)


def _build_nc(layout, ba):
    L_RUN = int(os.environ.get("KGAT_LAYERS", str(L)))
    TOTCH = layout["TOTCH"]
    regions = layout["regions"]
    win_chunks = layout["win_chunks"]
    dt = mybir.dt
    nc = bacc.Bacc("TRN2", target_bir_lowering=False, debug=False,
                   enable_asserts=False, num_devices=NCORES)

    i_xslab = nc.dram_tensor("xslab", [NSLAB, ROWC], dt.bfloat16, kind="ExternalInput")
    i_xt0 = nc.dram_tensor("xt0", [P, NSLAB], dt.bfloat16, kind="ExternalInput")
    i_t0 = nc.dram_tensor("t0", [P, WPC], dt.float32, kind="ExternalInput")
    i_idx16 = nc.dram_tensor("idx16", [P, TOTCH * 8], dt.int16, kind="ExternalInput")
    i_dla = nc.dram_tensor("dla", [P, TOTCH], dt.float32, kind="ExternalInput")
    i_wg = nc.dram_tensor("wg", [L, 2, D, D], dt.bfloat16, kind="ExternalInput")
    i_wast = nc.dram_tensor("wast", [L - 1, D, 2], dt.bfloat16, kind="ExternalInput")
    i_bg = nc.dram_tensor("bg", [L, D, 1], dt.float32, kind="ExternalInput")
    o_out = nc.dram_tensor("out", [NSLAB, D], dt.float32, kind="ExternalOutput")

    agin = [nc.dram_tensor(f"agin{l}", [NSLAB, ROWC], dt.bfloat16, kind="Internal")
            for l in range(L)]
    xfull = [nc.dram_tensor(f"xfull{l}", [TAB, ROWC], dt.bfloat16, kind="Internal",
                            addr_space="Shared")
             for l in range(L)]

    with tile.TileContext(nc) as tc:
        with (
            tc.tile_pool(name="sb", bufs=1) as sb,
            tc.tile_pool(name="sbg", bufs=2) as sbg,
            tc.tile_pool(name="sbw", bufs=3) as sbw,
            tc.tile_pool(name="ps", bufs=2, space="PSUM") as ps,
            tc.tile_pool(name="ps1", bufs=1, space="PSUM") as ps1,
            tc.tile_pool(name="psT", bufs=1, space="PSUM") as psT,
        ):
            # ---- constants / persistent state ----
            iota_i = sb.tile([P, P], dt.int32)
            nc.gpsimd.iota(iota_i[:], pattern=[[1, P]], base=0, channel_multiplier=0)
            iota_b = sb.tile([P, P], dt.bfloat16)
            nc.vector.tensor_copy(out=iota_b[:], in_=iota_i[:])
            iotac_i = sb.tile([P, 1], dt.int32)
            nc.gpsimd.iota(iotac_i[:], pattern=[[0, 1]], base=0, channel_multiplier=1)
            iotac_f = sb.tile([P, 1], dt.float32)
            nc.vector.tensor_copy(out=iotac_f[:], in_=iotac_i[:])
            ones_b = sb.tile([P, P], dt.bfloat16)
            nc.vector.memset(ones_b[:], 1.0)
            ident_b = sb.tile([P, P], dt.bfloat16)
            make_identity(nc, ident_b[:])
            ident_f = sb.tile([P, P], dt.float32)
            make_identity(nc, ident_f[:])

            idx16_sb = sb.tile([P, TOTCH * 8], dt.int16)
            nc.sync.dma_start(out=idx16_sb[:], in_=i_idx16.ap())
            dla_sb = sb.tile([P, TOTCH], dt.float32)
            nc.sync.dma_start(out=dla_sb[:], in_=i_dla.ap())

            wg_sb = sb.tile([P, L * 2 * D], dt.bfloat16)
            for l in range(L):
                for h in range(2):
                    nc.sync.dma_start(out=wg_sb[:, (l * 2 + h) * D:(l * 2 + h + 1) * D],
                                      in_=i_wg.ap()[l, h])
            wast_sb = sb.tile([P, (L - 1) * 2], dt.bfloat16)
            for l in range(L - 1):
                nc.sync.dma_start(out=wast_sb[:, l * 2:l * 2 + 2], in_=i_wast.ap()[l])
            bg_sb = sb.tile([P, L], dt.float32)
            for l in range(L):
                nc.sync.dma_start(out=bg_sb[:, l:l + 1], in_=i_bg.ap()[l])

            xt_own = sb.tile([P, NSLAB], dt.bfloat16)
            nc.sync.dma_start(out=xt_own[:], in_=i_xt0.ap())

            tstages = [sb.tile([P, WPC], dt.float32, tag=f"tst{l}", name=f"tst{l}")
                       for l in range(L)]
            nc.sync.dma_start(out=tstages[0][:], in_=i_t0.ap())

            NO_COLL = int(os.environ.get("KGAT_NO_COLL", "0"))

            def allgather(src_t, dst_t):
                if NO_COLL:
                    for k in range(NCORES):
                        nc.sync.dma_start(
                            out=dst_t.ap()[k * NSLAB:(k + 1) * NSLAB], in_=src_t.ap())
                else:
                    nc.gpsimd.collective_compute(
                        "AllGather", mybir.AluOpType.bypass,
                        replica_groups=[list(range(NCORES))],
                        ins=[src_t.ap()], outs=[dst_t.ap()])

            # replicate the layer-0 table
            nc.sync.dma_start(out=agin[0].ap(), in_=i_xslab.ap())
            allgather(agin[0], xfull[0])

            for l in range(L_RUN):
                last = (l == L_RUN - 1)
                xsrc = xfull[l]
                tst = tstages[l]
                if not last:
                    stage = sb.tile([P, WPC, ROWC], dt.bfloat16, tag="stage")
                    nc.vector.memset(stage[:, :, D + 1:], 0)
                else:
                    stagef = sb.tile([P, WPC, D], dt.float32, tag="stage")

                for reg in regions:
                    ch0, ch1 = reg["ch0"], reg["ch1"]
                    G = sbg.tile([P, ch1 - ch0, ROWC], dt.bfloat16, tag="G")
                    for (q, cha, chb) in reg["q_ranges"]:
                        nidx = (chb - cha) * P
                        nc.gpsimd.dma_gather(
                            G[:, cha - ch0:chb - ch0, :],
                            xsrc.ap()[q * QROWS:(q + 1) * QROWS],
                            idx16_sb[:, cha * 8:chb * 8],
                            nidx, nidx, ROWC, single_packet=False)

                    for w in range(reg["w0"], reg["w1"]):
                        chunks = win_chunks[w]
                        # T_bc[e, j] = t_w[j] broadcast: ones^T @ diag(t_w)
                        diag = sbw.tile([P, P], dt.bfloat16, tag="diag")
                        nc.vector.tensor_scalar(
                            diag[:], iota_b[:], iotac_f[:, 0:1], tst[:, w:w + 1],
                            mybir.AluOpType.is_equal, mybir.AluOpType.mult)
                        tbc = psT.tile([P, P], dt.float32, tag="tbc")
                        nc.tensor.matmul(out=tbc[:], lhsT=ones_b[:], rhs=diag[:],
                                         start=True, stop=True)

                        aggp = ps.tile([P, P], dt.float32, tag="agg")
                        nch = len(chunks)
                        for i, gc in enumerate(chunks):
                            c = gc - ch0
                            OH = sbw.tile([P, P], dt.bfloat16, tag="OH")
                            nc.vector.tensor_scalar(
                                OH[:], iota_b[:], dla_sb[:, gc:gc + 1], None,
                                mybir.AluOpType.is_equal)
                            SIG = sbw.tile([P, P], dt.bfloat16, tag="SIG")
                            nc.scalar.activation(
                                out=SIG[:], in_=tbc[:],
                                func=mybir.ActivationFunctionType.Sigmoid,
                                bias=G[:, c, D:D + 1])
                            Wt = sbw.tile([P, P], dt.bfloat16, tag="W")
                            nc.vector.tensor_tensor(
                                out=Wt[:], in0=OH[:], in1=SIG[:],
                                op=mybir.AluOpType.mult)
                            nc.tensor.matmul(out=aggp[:], lhsT=G[:, c, 0:D],
                                             rhs=Wt[:],
                                             start=(i == 0), stop=(i == nch - 1))

                        # ---- node update for window w ----
                        aggb = sbw.tile([P, P], dt.bfloat16, tag="aggb")
                        nc.vector.tensor_copy(out=aggb[:], in_=aggp[:])
                        xts = xt_own[:, w * P:(w + 1) * P]
                        up = ps.tile([P, P], dt.float32, tag="up")
                        nc.tensor.matmul(out=up[:],
                                         lhsT=wg_sb[:, (l * 2) * D:(l * 2 + 1) * D],
                                         rhs=xts, start=True, stop=False)
                        nc.tensor.matmul(out=up[:],
                                         lhsT=wg_sb[:, (l * 2 + 1) * D:(l * 2 + 2) * D],
                                         rhs=aggb[:], start=False, stop=True)
                        if not last:
                            nc.scalar.activation(out=xts, in_=up[:],
                                                 func=mybir.ActivationFunctionType.Relu,
                                                 bias=bg_sb[:, l:l + 1])
                            st = ps1.tile([P, 2], dt.float32, tag="st")
                            nc.tensor.matmul(out=st[:], lhsT=xts,
                                             rhs=wast_sb[:, l * 2:l * 2 + 2],
                                             start=True, stop=True)
                            tr = ps1.tile([P, P], dt.bfloat16, tag="tr")
                            nc.tensor.transpose(out=tr[:], in_=xts, identity=ident_b[:])
                            nc.vector.tensor_copy(out=stage[:, w, 0:D], in_=tr[:])
                            nc.scalar.add(out=stage[:, w, D:D + 1], in_=st[:, 0:1],
                                          add=float(ba[l + 1, 0]))
                            nc.vector.tensor_copy(out=tstages[l + 1][:, w:w + 1],
                                                  in_=st[:, 1:2])
                        else:
                            xf = sbw.tile([P, P], dt.float32, tag="xf")
                            nc.scalar.activation(out=xf[:], in_=up[:],
                                                 func=mybir.ActivationFunctionType.Relu,
                                                 bias=bg_sb[:, l:l + 1])
                            trf = ps1.tile([P, P], dt.float32, tag="trf")
                            nc.tensor.transpose(out=trf[:], in_=xf[:], identity=ident_f[:])
                            nc.vector.tensor_copy(out=stagef[:, w, :], in_=trf[:])

                if not last:
                    nc.sync.dma_start(
                        out=agin[l + 1].ap().rearrange("(w p) c -> p w c", p=P),
                        in_=stage[:])
                    allgather(agin[l + 1], xfull[l + 1])
                else:
                    nc.sync.dma_start(
                        out=o_out.ap().rearrange("(w p) c -> p w c", p=P),
                        in_=stagef[:])

    nc.compile()
    return nc


def kernel(edge_index, user_emb, item_emb, Wa, ba, Wg, bg):
    global LAST_EXEC_NS
    h = _host_prep(edge_index, user_emb, item_emb, Wa, ba, Wg, bg)
    nc = _build_nc(h["layout"], h["ba"])

    in_maps = []
    for k in range(NCORES):
        in_maps.append({
            "xslab": h["xslab"][k], "xt0": h["xt0"][k], "t0": h["t0c"][k],
            "idx16": h["idx16"][k], "dla": h["dla"][k],
            "wg": h["wg_b"], "wast": h["wast"], "bg": h["bg_c"],
        })

    res = run_bass_kernel_spmd(nc, in_maps, core_ids=list(range(NCORES)))
    LAST_EXEC_NS = res.exec_time_ns

    if int(os.environ.get("KGAT_BENCH", "0")):
        LAST_EXEC_NS = _bench(nc, in_maps)

    x = np.zeros((N, D), np.float32)
    for k in range(NCORES):
        x[k * NPC:(k + 1) * NPC] = np.asarray(res.results[k]["out"])[:NPC]
    return x[:U], x[U:]


def _bench(nc, in_maps, iters=None):
    """Time repeated on-device executions via the same PJRT shard_map path
    (device-resident inputs, no donation) and return min wall ns."""
    import time
    import jax
    from jax.sharding import Mesh, PartitionSpec
    from jax.experimental.shard_map import shard_map
    from concourse import bass2jax, mybir as mb

    if iters is None:
        iters = int(os.environ.get("KGAT_BENCH_ITERS", "10"))

    bass2jax.install_neuronx_cc_hook()
    partition_name = (nc.partition_id_tensor.name
                      if nc.partition_id_tensor else None)
    in_names, out_names, out_avals, zero_outs = [], [], [], []
    for alloc in nc.m.functions[0].allocations:
        if not isinstance(alloc, mb.MemoryLocationSet):
            continue
        name = alloc.memorylocations[0].name
        if alloc.kind == "ExternalInput":
            if name != partition_name:
                in_names.append(name)
        elif alloc.kind == "ExternalOutput":
            out_names.append(name)
            shape = tuple(alloc.tensor_shape)
            dtype = mb.dt.np(alloc.dtype)
            out_avals.append(jax.core.ShapedArray(shape, dtype))
            zero_outs.append(np.zeros(shape, dtype))
    n_params = len(in_names)
    all_names = in_names + out_names
    if partition_name is not None:
        all_names = all_names + [partition_name]

    def _body(*args):
        operands = list(args)
        if partition_name is not None:
            operands.append(bass2jax.partition_id_tensor())
        return tuple(bass2jax._bass_exec_p.bind(
            *operands, out_avals=tuple(out_avals), in_names=tuple(all_names),
            out_names=tuple(out_names), lowering_input_output_aliases=(),
            sim_require_finite=False, sim_require_nnan=False, nc=nc))

    devices = jax.devices()[:NCORES]
    mesh = Mesh(np.asarray(devices), ("core",))
    specs = (PartitionSpec("core"),) * (n_params + len(out_names))
    fn = jax.jit(shard_map(_body, mesh=mesh, in_specs=specs,
                           out_specs=(PartitionSpec("core"),) * len(out_names),
                           check_rep=False), keep_unused=True)
    concat_in = [np.concatenate([np.asarray(m[n]) for m in in_maps], axis=0)
                 for n in in_names]
    concat_zero = [np.zeros((NCORES * z.shape[0], *z.shape[1:]), z.dtype)
                   for z in zero_outs]
    sharding = jax.sharding.NamedSharding(mesh, PartitionSpec("core"))
    dev_in = [jax.device_put(a, sharding) for a in concat_in + concat_zero]
    jax.block_until_ready(fn(*dev_in))  # warm compile
    times = []
    for _ in range(iters):
        t0 = time.perf_counter()
        jax.block_until_ready(fn(*dev_in))
        times.append(time.perf_counter() - t0)
    times.sort()
    print(f"bench iters (ms): {[f'{t*1e3:.2f}' for t in times]}")
    return int(times[0] * 1e9)
